# revision 1
# baseline (speedup 1.0000x reference)
"""Self-contained TRN2 Bass kernel: 2-layer LSTM classifier (nn_CustomLSTMClassifier).

Reference model: x[512,1024,64] -> 2-layer LSTM (H=128) -> logits[512,4].

Strategy: data-parallel over the 8 NeuronCores -- the batch of 512 is split
into 8 shards of 64; the small weights (~0.4 MB) are replicated.  Each core
runs the full 1024-step recurrence for its shard; there are no collectives.

Per-core design (states as [H=128 partitions, B=64 free] tiles):
  - Gates computed transposed, gates.T [4H, B], as 4 chunks of [128, B] in one
    PSUM tile.  Layer-1's input projection (+bias via a ones-row, K=65) is
    batched over 4-step blocks straight into PSUM; the per-step recurrent
    projections are uniform K=128 matmuls that accumulate in place.
  - Everything is sigmoid-only: tanh(g) = 2*sigmoid(2g)-1 with the doubling
    folded into the g-gate weight rows on the host; the recurrent state is
    kept as h~ = h/2 = (sigmoid(2c)-0.5)*sigmoid(o) (one fused DVE op), with
    the consuming matrices (Wh1, Wx2, Wh2, Wc) pre-doubled on the host.
  - Cell math uses fused scalar_tensor_tensor ops; cell state c stays fp32,
    matmul operands are bf16.
  - Layer-2 bias is delivered by a K=4 "indicator" matmul that also opens
    (clears) the PSUM accumulation bank.
"""

import os

import numpy as np
import ml_dtypes

import concourse.bass as bass
import concourse.mybir as mybir
import concourse.tile as tile
from concourse.tile import TileContext
from concourse.bass_utils import run_bass_kernel_spmd

F32 = mybir.dt.float32
BF16 = mybir.dt.bfloat16
AF = mybir.ActivationFunctionType
OP = mybir.AluOpType

P = 128          # hidden size == partition count
B = 64           # batch shard per core
IN = 64          # input size
INA = IN + 1     # augmented with ones row (bias1)
NG = 4           # gates
SEQ = 1024
BATCH = 512
NCORES = 8
TBX = 4          # steps per xg PSUM block (2 gate chunks per PSUM bank)
SLAB = 32        # steps per x DMA slab

_wsplit_counter = [0]


def _split_excess_waits(nc, max_waits=1):
    """This walrus build rejects instructions carrying more than `max_waits`
    sem waits.  Move the excess onto NoOp carriers inserted just before the
    instruction on the same engine stream (sequential, so semantics hold)."""
    n_split = 0
    for fn in nc.m.functions:
        for blk in fn.blocks:
            out = []
            changed = False
            for inst in blk.instructions:
                si = inst.sync_info
                waits = list(si.on_wait) if si is not None and si.on_wait else []
                if len(waits) > max_waits:
                    changed = True
                    n_split += 1
                    extras, keep = waits[:-max_waits], waits[-max_waits:]
                    for i in range(0, len(extras), max_waits):
                        chunk = extras[i : i + max_waits]
                        _wsplit_counter[0] += 1
                        nop = mybir.InstNoOp(
                            name=f"wsplit-{_wsplit_counter[0]}", ins=[], outs=[]
                        )
                        nop.engine = inst.engine
                        nop.sync_info = mybir.SyncInfo(on_wait=chunk, on_update=[])
                        out.append(nop)
                    si.on_wait = keep
                out.append(inst)
            if changed:
                blk.instructions = out
    return n_split


def _build_lstm(seq=SEQ):
    nc = bass.Bass()

    nslab = seq // SLAB
    xT = nc.declare_dram_parameter("xT", [nslab, INA, SLAB * B], BF16, isOutput=False)
    wx1 = nc.declare_dram_parameter("wx1", [INA, NG * P], BF16, isOutput=False)
    wh1 = nc.declare_dram_parameter("wh1", [P, NG * P], BF16, isOutput=False)
    wx2 = nc.declare_dram_parameter("wx2", [P, NG * P], BF16, isOutput=False)
    wh2 = nc.declare_dram_parameter("wh2", [P, NG * P], BF16, isOutput=False)
    b2m = nc.declare_dram_parameter("b2m", [NG, P], BF16, isOutput=False)
    ind = nc.declare_dram_parameter("ind", [NG, NG * B], BF16, isOutput=False)
    wcT = nc.declare_dram_parameter("wcT", [P, 4], BF16, isOutput=False)
    bc = nc.declare_dram_parameter("bc", [4, 1], F32, isOutput=False)
    out = nc.declare_dram_parameter("logitsT", [4, B], F32, isOutput=True)

    with TileContext(nc) as tc:
        with (
            tc.tile_pool(name="consts", bufs=1) as cw,
            tc.tile_pool(name="xpool", bufs=2) as xpool,
            tc.tile_pool(name="state", bufs=3) as st,
            tc.tile_pool(name="work", bufs=3) as wk,
            tc.tile_pool(name="pxg", bufs=2, space="PSUM") as pxg,
            tc.tile_pool(name="pg2", bufs=2, space="PSUM") as pg2p,
            tc.tile_pool(name="pout", bufs=1, space="PSUM") as poutp,
        ):
            t_wx1 = cw.tile([INA, NG * P], BF16, tag="wx1")
            nc.sync.dma_start(t_wx1[:], wx1[:])
            t_wh1 = cw.tile([P, NG * P], BF16, tag="wh1")
            nc.sync.dma_start(t_wh1[:], wh1[:])
            t_wx2 = cw.tile([P, NG * P], BF16, tag="wx2")
            nc.sync.dma_start(t_wx2[:], wx2[:])
            t_wh2 = cw.tile([P, NG * P], BF16, tag="wh2")
            nc.sync.dma_start(t_wh2[:], wh2[:])
            t_b2m = cw.tile([NG, P], BF16, tag="b2m")
            nc.sync.dma_start(t_b2m[:], b2m[:])
            t_ind = cw.tile([NG, NG * B], BF16, tag="ind")
            nc.sync.dma_start(t_ind[:], ind[:])
            t_wcT = cw.tile([P, 4], BF16, tag="wcT")
            nc.sync.dma_start(t_wcT[:], wcT[:])
            t_bc = cw.tile([4, 1], F32, tag="bc")
            nc.sync.dma_start(t_bc[:], bc[:])

            h1 = st.tile([P, B], BF16, tag="h1")
            nc.vector.memset(h1[:], 0)
            h2 = st.tile([P, B], BF16, tag="h2")
            nc.vector.memset(h2[:], 0)
            c1 = st.tile([P, B], F32, tag="c1")
            nc.vector.memset(c1[:], 0)
            c2 = st.tile([P, B], F32, tag="c2")
            nc.vector.memset(c2[:], 0)

            xslabs = {}

            def batch_xproj(xg_t, blk):
                """Batched x-projection (K=65, incl. bias row) for one block.
                Chunks 0,1 share PSUM bank 0 and chunks 2,3 bank 1: the even
                chunk opens its bank (start=True), the odd chunk relies on
                first-touch-overwrite and is dep-ordered after the opener."""
                sl, sb = divmod(blk, SLAB // TBX)
                if sl not in xslabs:
                    xs_t = xpool.tile([INA, SLAB * B], BF16, tag="xslab")
                    nc.sync.dma_start(xs_t[:], xT[sl])
                    xslabs[sl] = xs_t
                xs = xslabs[sl][:, sb * TBX * B : (sb + 1) * TBX * B]
                opener = None
                for m in range(NG):
                    mm = nc.tensor.matmul(
                        xg_t[:, m, :],
                        t_wx1[:, m * P : (m + 1) * P],
                        xs,
                        start=(m % 2 == 0),
                        stop=False,
                        skip_group_check=True,
                    )
                    if m % 2 == 0:
                        opener = mm
                    else:
                        tile.add_dep_helper(
                            mm.ins, opener.ins, sync=False, reason="bank open order"
                        )

            def hproj1(xg_t, js_t):
                for m in range(NG):
                    nc.tensor.matmul(
                        xg_t[:, m, js_t],
                        t_wh1[:, m * P : (m + 1) * P],
                        h1[:],
                        start=False,
                        stop=True,
                        skip_group_check=True,
                    )

            def cell(sig, c_old, lyr):
                """sig: [P, 4B] bf16 = sigmoid over [i,f,o,2g] preacts.
                Returns (h~_new, c_new) with h~ = h/2 = (sig(2c)-0.5)*sig(o)."""
                u = wk.tile([P, B], BF16, tag=f"u{lyr}")
                nc.vector.scalar_tensor_tensor(
                    u[:], sig[:, 3 * B : 4 * B], 0.5, sig[:, 0:B],
                    OP.subtract, OP.mult,
                )
                t2 = wk.tile([P, B], F32, tag=f"t2{lyr}")
                nc.vector.tensor_tensor(t2[:], sig[:, B : 2 * B], c_old[:], OP.mult)
                cn = st.tile([P, B], F32, tag=f"c{lyr}")
                nc.vector.scalar_tensor_tensor(cn[:], u[:], 2.0, t2[:], OP.mult, OP.add)
                sc = wk.tile([P, B], F32, tag=f"sc{lyr}")
                nc.scalar.activation(sc[:], cn[:], AF.Sigmoid, scale=2.0)
                hn = st.tile([P, B], BF16, tag=f"h{lyr}")
                nc.vector.scalar_tensor_tensor(
                    hn[:], sc[:], 0.5, sig[:, 2 * B : 3 * B], OP.subtract, OP.mult
                )
                return hn, cn

            xg_cur = pxg.tile([P, NG, TBX * B], F32, tag="xg")
            batch_xproj(xg_cur, 0)
            hproj1(xg_cur, slice(0, B))
            xg_next = None

            for t in range(seq):
                blk, j = divmod(t, TBX)
                js = slice(j * B, (j + 1) * B)
                if j == 0 and t > 0:
                    xg_cur = xg_next

                # layer-2 bias/opener matmul: no data deps, fills PE idle time
                pg2 = pg2p.tile([P, NG * B], F32, tag="pg2")
                nc.tensor.matmul(
                    pg2[:], t_b2m[:], t_ind[:], start=True, stop=False,
                    skip_group_check=True,
                )

                # layer 1 (sigma waits only on this step's 4 hproj1 matmuls)
                sig1 = wk.tile([P, NG * B], BF16, tag="sig1")
                nc.scalar.activation(sig1[:], xg_cur[:, :, js], AF.Sigmoid)
                h1, c1 = cell(sig1, c1, "1")

                # PE work gated on h~1(t): next step's hproj1 first (it alone
                # gates sigma1(t+1)), then layer-2's 8 uniform K=128 matmuls
                if t + 1 < seq:
                    nblk, nj = divmod(t + 1, TBX)
                    if nj == 0:
                        xg_next = pxg.tile([P, NG, TBX * B], F32, tag="xg")
                        batch_xproj(xg_next, nblk)
                        hproj1(xg_next, slice(0, B))
                    else:
                        hproj1(xg_cur, slice(nj * B, (nj + 1) * B))
                for m in range(NG):
                    cs = slice(m * B, (m + 1) * B)
                    ws = slice(m * P, (m + 1) * P)
                    nc.tensor.matmul(
                        pg2[:, cs], t_wx2[:, ws], h1[:], start=False, stop=False,
                        skip_group_check=True,
                    )
                    nc.tensor.matmul(
                        pg2[:, cs], t_wh2[:, ws], h2[:], start=False,
                        stop=(m == NG - 1), skip_group_check=True,
                    )
                sig2 = wk.tile([P, NG * B], BF16, tag="sig2")
                nc.scalar.activation(sig2[:], pg2[:], AF.Sigmoid)
                h2, c2 = cell(sig2, c2, "2")

            # classifier head (Wc pre-doubled for h~)
            pl = poutp.tile([4, B], F32, tag="pl")
            nc.tensor.matmul(pl[:], t_wcT[:], h2[:], start=True, stop=True)
            lt = wk.tile([4, B], F32, tag="lt")
            nc.scalar.activation(lt[:], pl[:], AF.Identity, bias=t_bc[:])
            nc.sync.dma_start(out[:], lt[:])

    _split_excess_waits(nc, max_waits=1)
    return nc


# ---------------- host-side preprocessing ----------------

def _permute_gates(w, scale_g=True):
    """Reorder gate-major rows [i,f,g,o] -> [i,f,o,g]; scale g rows by 2
    (tanh(g) = 2*sigmoid(2g) - 1)."""
    i, f, g, o = np.split(np.asarray(w, dtype=np.float64), 4, axis=0)
    if scale_g:
        g = 2.0 * g
    return np.concatenate([i, f, o, g], axis=0)


def _prep_weights(Wx1, bx1, Wh1, bh1, Wx2, bx2, Wh2, bh2, Wc, bc):
    """Pack replicated weights.  Matrices consuming the halved recurrent state
    h~ = h/2 (Wh1, Wx2, Wh2, Wc) are doubled."""
    b1 = _permute_gates((np.asarray(bx1) + np.asarray(bh1))[:, None])[:, 0]
    wx1 = _permute_gates(Wx1).T
    wx1a = np.concatenate([wx1, b1[None, :]], axis=0)  # [IN+1, 4H]
    wh1 = 2.0 * _permute_gates(Wh1).T
    wx2 = 2.0 * _permute_gates(Wx2).T
    wh2 = 2.0 * _permute_gates(Wh2).T
    b2 = _permute_gates((np.asarray(bx2) + np.asarray(bh2))[:, None])[:, 0]
    b2m = b2.reshape(NG, P)
    ind = np.zeros((NG, NG * B), dtype=np.float32)
    for k in range(NG):
        ind[k, k * B : (k + 1) * B] = 1.0
    bf = ml_dtypes.bfloat16
    return {
        "wx1": wx1a.astype(bf),
        "wh1": wh1.astype(bf),
        "wx2": wx2.astype(bf),
        "wh2": wh2.astype(bf),
        "b2m": b2m.astype(bf),
        "ind": ind.astype(bf),
        "wcT": (2.0 * np.asarray(Wc, dtype=np.float64).T).astype(bf),
        "bc": np.asarray(bc).reshape(4, 1).astype(np.float32),
    }


def _prep_x_shard(x_shard):
    """[B, S, IN] fp32 -> [S//SLAB, IN+1, SLAB*B] bf16; slab columns ordered
    (t_in_slab, b); row IN is the ones row carrying bias1."""
    Bs, S, I = x_shard.shape
    xt = np.ascontiguousarray(np.transpose(np.asarray(x_shard), (1, 2, 0)))
    xt = xt.reshape(S // SLAB, SLAB, I, Bs).transpose(0, 2, 1, 3)
    xt = np.ascontiguousarray(xt).reshape(S // SLAB, I, SLAB * Bs)
    ones = np.ones((S // SLAB, 1, SLAB * Bs), dtype=xt.dtype)
    return np.concatenate([xt, ones], axis=1).astype(ml_dtypes.bfloat16)


_NC_CACHE = {}


def kernel(**inputs) -> np.ndarray:
    x = np.asarray(inputs["x"], dtype=np.float32)
    assert x.shape == (BATCH, SEQ, IN), x.shape

    if "nc" not in _NC_CACHE:
        _NC_CACHE["nc"] = _build_lstm(SEQ)
    nc = _NC_CACHE["nc"]

    wmaps = _prep_weights(
        inputs["Wx1"], inputs["bx1"], inputs["Wh1"], inputs["bh1"],
        inputs["Wx2"], inputs["bx2"], inputs["Wh2"], inputs["bh2"],
        inputs["Wc"], inputs["bc"],
    )
    in_maps = []
    for c in range(NCORES):
        m = dict(wmaps)
        m["xT"] = _prep_x_shard(x[c * B : (c + 1) * B])
        in_maps.append(m)

    trace = os.environ.get("LSTM_BASS_TRACE") == "1"
    res = run_bass_kernel_spmd(nc, in_maps, list(range(NCORES)), trace=trace)
    if trace:
        kernel.last_exec_time_ns = res.exec_time_ns

    logits = np.concatenate(
        [res.results[i]["logitsT"].T for i in range(NCORES)], axis=0
    ).astype(np.float32)
    return logits

